# revision 3
# baseline (speedup 1.0000x reference)
"""Frequency-Channel-Attention kernel for Trainium2 (8 NeuronCores, SPMD), v2.

Math: dct2(X) = D @ X @ D^T with D[k,j] = cos(pi*k*(2j+1)/(2L))/L, L=64.
Per (b,c): S = max(dct2(ip[b,c])); h = relu(S@w1); z = sigmoid(h@w2);
out = ip * z[b,c].

v2 dataflow (per core, 2 batches x 256 channels; 16KB-descriptor DMA +
full-array matmuls):
  - Load ip f32 image-per-partition: T16f[c%128, (j, h*w)], j = 2*b + cg,
    cg = c//128.  128 descriptors x 16KB per call -> ~350GB/s.
  - Cast to bf16 chunks; XBAR DMA-transpose converts each chunk to
    X[pi=(r,w), (hbar, c)] with h = 2*hbar + r (one DMA instr per chunk;
    FCA_CONV_MODE=pe falls back to PE transposes).
  - Stage 1 (contract w): lhsT = BD1[(r,w),(r',k2)] = d_rr' D[k2,w],
    K=128 M=128 N=512 -> A'[(r',k2), (hbar, c)]; copy to bf16 A with free
    reordered to (c, hbar).
  - Middle transpose on PE: [128,128] blocks over (j-pair, c-pair, hbar)
    -> PSUM bf16, copied to R[(jhat,chat,hbar), (t4, r', k2)].
  - Stage 2 (contract h=(hbar,r')): lhsT = BD2 zero-block selecting
    (jhat', r'), 2 accumulating passes -> dct^T[(chat',k1), (t,k2)].
  - reduce_max over k2 -> M1[(chat',k1), 128*b + 64*cg + pair]; per-batch
    PE transpose + reduce -> S_b; MLP on PE; z^T via W2-chunk-stationary
    matmuls + sigmoid -> Zpp[c%128, j].
  - In-place multiply T16f *= Zpp (broadcast over h*w), store with 8KB
    descriptors.
"""

import os
import sys

import numpy as np

for _p in ("/opt/trn_rl_repo", "/opt/pypackages"):
    if os.path.isdir(_p) and _p not in sys.path:
        sys.path.append(_p)

import concourse.bacc as bacc
import concourse.tile as tile
from concourse import mybir
from concourse.bass_utils import run_bass_kernel_spmd

F32 = mybir.dt.float32
BF16 = mybir.dt.bfloat16

B, C, H, W = 16, 256, 64, 64
N_CORES = 8
B_LOC = B // N_CORES
NJ = 4  # j = 2*b + cg

_NC_CACHE = {}
CONV_MODE = os.environ.get("FCA_CONV_MODE", "xbar")


def _dct_matrix():
    k = np.arange(W, dtype=np.float64)[:, None]
    j = np.arange(W, dtype=np.float64)[None, :]
    D = np.cos(np.pi * k * (2.0 * j + 1.0) / (2.0 * W)) / W
    return D.astype(np.float32)


def _constants():
    D = _dct_matrix()
    BD1 = np.zeros((128, 128), dtype=np.float32)
    for r in range(2):
        BD1[64 * r : 64 * r + 64, 64 * r : 64 * r + 64] = D.T
    BD2 = np.zeros((128, 512), dtype=np.float32)
    for jh in range(2):
        for ch in range(2):
            for rp in range(2):
                blk = D.T[rp::2, :]  # blk[hbar, k1] = D[k1, 2hbar+rp]
                BD2[
                    64 * ch + 32 * jh : 64 * ch + 32 * jh + 32,
                    128 * (2 * jh + rp) + 64 * ch : 128 * (2 * jh + rp)
                    + 64 * ch
                    + 64,
                ] = blk
    identf = np.eye(128, dtype=np.float32)
    return BD1, BD2, identf


def _build_nc(conv_mode):
    nc = bacc.Bacc(None, target_bir_lowering=False)
    ip_d = nc.dram_tensor("ip", [B_LOC, C, H, W], F32, kind="ExternalInput")
    w1e_d = nc.dram_tensor("w1e", [128, 16], F32, kind="ExternalInput")
    w1o_d = nc.dram_tensor("w1o", [128, 16], F32, kind="ExternalInput")
    w2_d = nc.dram_tensor("w2", [16, C], F32, kind="ExternalInput")
    bd1_d = nc.dram_tensor("bd1", [128, 128], F32, kind="ExternalInput")
    bd2_d = nc.dram_tensor("bd2", [128, 512], F32, kind="ExternalInput")
    idf_d = nc.dram_tensor("identf", [128, 128], F32, kind="ExternalInput")
    out_d = nc.dram_tensor("out", [B_LOC, C, H, W], F32, kind="ExternalOutput")

    from contextlib import ExitStack

    with tile.TileContext(nc) as tc, ExitStack() as ctx:
        const = ctx.enter_context(tc.tile_pool(name="const", bufs=1))
        big = ctx.enter_context(tc.tile_pool(name="big", bufs=1))
        chk = ctx.enter_context(tc.tile_pool(name="chk", bufs=2))
        rpool = ctx.enter_context(tc.tile_pool(name="rp", bufs=3))
        misc = ctx.enter_context(tc.tile_pool(name="misc", bufs=1))
        psap = ctx.enter_context(tc.tile_pool(name="psa", bufs=2, space="PSUM"))
        psbp = ctx.enter_context(tc.tile_pool(name="psb", bufs=2, space="PSUM"))
        ps2p = ctx.enter_context(tc.tile_pool(name="ps2", bufs=2, space="PSUM"))

        def load_const(name_d, shape, tag):
            t = const.tile(shape, F32, tag=tag)
            nc.scalar.dma_start(out=t, in_=name_d[:, :])
            return t

        BD1f = load_const(bd1_d, [128, 128], "bd1f")
        BD2f = load_const(bd2_d, [128, 512], "bd2f")
        IDTf = load_const(idf_d, [128, 128], "idf")
        W1E = load_const(w1e_d, [128, 16], "w1e")
        W1O = load_const(w1o_d, [128, 16], "w1o")
        W2t = load_const(w2_d, [16, 256], "w2t")
        BD1 = const.tile([128, 128], BF16)
        nc.scalar.copy(out=BD1, in_=BD1f)
        BD2 = const.tile([128, 512], BF16)
        nc.scalar.copy(out=BD2, in_=BD2f)
        IDTb = const.tile([128, 128], BF16)
        nc.scalar.copy(out=IDTb, in_=IDTf)

        HWE = [nc.sync, nc.scalar]  # HWDGE-capable engines

        _cp = [0]

        def copy_psum(i, out, in_):
            # PSUM readers: ACT + DVE only (gpsimd cannot access PSUM).
            # ~2:1 ACT:DVE split (DVE also owns the reduces).
            _cp[0] += 1
            if _cp[0] % 3 == 0:
                nc.vector.tensor_copy(out=out, in_=in_)
            else:
                nc.scalar.copy(out=out, in_=in_)

        def vg(i):
            return nc.gpsimd if i % 4 != 3 else nc.vector

        # ---- full f32 input, image-per-partition ----
        T16f = big.tile([128, NJ * H * W], F32)
        ip_v = ip_d.rearrange("b (cg cl) h w -> cl b cg (h w)", cg=2)
        ip_q = ip_d.rearrange("b (cg cl) (hh hw) w -> cl b cg hh (hw w)", cg=2, hh=4)
        for j in range(NJ):
            b, cg = j // 2, j % 2
            nq = 4 if j < 2 else 2
            for q in range(nq):
                step = 4096 // nq
                HWE[q % 2].dma_start(
                    out=T16f[:, H * W * j + step * q : H * W * j + step * (q + 1)],
                    in_=ip_q.rearrange("cl b cg hh x -> cl b cg (hh x)")[
                        :, b, cg, step * q : step * (q + 1)
                    ],
                )

        A = big.tile([128, NJ * 128 * 32], BF16)
        Av = A.rearrange("p (a c jj hb) -> p a c jj hb", a=2, c=128, hb=32)
        M1 = misc.tile([128, 256], F32)
        Zpp = misc.tile([128, NJ], F32)

        def stage1(j):
            # conv: PE transposes T16f h-pair blocks (f32, 2cyc/row) ->
            # psa f32 (2 banks, 8 slots); PSUM->SBUF copy casts to bf16
            # X[pi, (hbar, c)].
            X = chk.tile([128, 32 * 128], BF16, tag="x")
            Xv = X.rearrange("p (hb c) -> p hb c", c=128)
            def conv_grp(n2):
                psc = psap.tile([128, 1024], F32, tag="psa")
                for q in range(8):
                    hb = 8 * n2 + q
                    src_t = T16[2 * j + hb // 16]
                    nc.tensor.transpose(
                        psc[:, 128 * q : 128 * q + 128],
                        src_t[:, 128 * (hb % 16) : 128 * (hb % 16) + 128],
                        IDTf,
                    )
                copy_psum(
                    j + n2,
                    Xv[:, 8 * n2 : 8 * n2 + 8, :],
                    psc.rearrange("p (q c) -> p q c", q=8),
                )

            def s1_grp(m2):
                ps1 = psap.tile([128, 1024], F32, tag="psa")
                for mm in range(2):
                    nc.tensor.matmul(
                        ps1[:, 512 * mm : 512 * mm + 512],
                        lhsT=BD1,
                        rhs=Xv[:, 8 * m2 + 4 * mm : 8 * m2 + 4 * mm + 4, :],
                        start=True,
                        stop=True,
                    )
                copy_psum(
                    m2,
                    Av[:, j // 2, :, j % 2, 8 * m2 : 8 * m2 + 8],
                    ps1.rearrange("p (hb c) -> p c hb", hb=8),
                )

            for n2 in range(4):
                conv_grp(n2)
                if n2 >= 1:
                    s1_grp(n2 - 1)
            s1_grp(3)

        def stage2(a, v):
            """v indexes a pair of 4-channel-pair groups: 8 midT transposes
            -> one psb bank -> R2 [128, (rp, t8, k2)] -> 4 s2 matmuls
            (2 jh x 2 rp accumulating, N=512) -> 2 reduces -> M1."""
            psb = psbp.tile([128, 1024], BF16, tag="midt")
            for t in range(8):
                u4 = 8 * v + t
                base = 8192 * a + 128 * u4
                nc.tensor.transpose(
                    psb[:, 128 * t : 128 * t + 128],
                    A[:, base : base + 128],
                    IDTb,
                )
            R = rpool.tile([128, 1024], BF16, tag="r")
            copy_psum(
                v,
                R.rearrange("p (rp t k) -> p t rp k", rp=2, k=64),
                psb.rearrange("p (t rp k) -> p t rp k", t=8, rp=2),
            )
            for jh in range(2):
                ps2 = ps2p.tile([128, 512], F32, tag="ps2")
                for rp in range(2):
                    nc.tensor.matmul(
                        ps2,
                        lhsT=BD2[
                            :, 128 * (2 * jh + rp) : 128 * (2 * jh + rp) + 128
                        ],
                        rhs=R[:, 512 * rp : 512 * rp + 512],
                        start=(rp == 0),
                        stop=(rp == 1),
                    )
                j = 2 * a + jh
                b, cg = j // 2, j % 2
                base = 128 * b + 64 * cg + 8 * v
                nc.vector.reduce_max(
                    out=M1[:, base : base + 8],
                    in_=ps2.rearrange("p (t k) -> p t k", k=64),
                    axis=mybir.AxisListType.X,
                )

        ST = misc.tile([128, 4], F32)
        hT = misc.tile([16, 2], F32)

        def phase_b(b):
            Mt = ps2p.tile([128, 512], F32, tag="ps2")
            tp = Mt[:, 0:128]
            nc.tensor.transpose(tp, M1[:, 128 * b : 128 * b + 128], IDTf)
            nc.vector.reduce_max(
                out=ST[:, 2 * b : 2 * b + 2],
                in_=tp.rearrange("p (c2 k) -> p c2 k", k=64),
                axis=mybir.AxisListType.X,
            )
            ph = Mt[0:16, 128:130]
            nc.tensor.matmul(
                ph[:, 0:1], lhsT=W1E, rhs=ST[:, 2 * b : 2 * b + 1],
                start=True, stop=False,
            )
            nc.tensor.matmul(
                ph[:, 0:1], lhsT=W1O, rhs=ST[:, 2 * b + 1 : 2 * b + 2],
                start=False, stop=True,
            )
            nc.scalar.activation(
                out=hT[:, b : b + 1], in_=ph[:, 0:1],
                func=mybir.ActivationFunctionType.Relu,
            )
            pz = Mt[:, 132:134]
            for cg in range(2):
                nc.tensor.matmul(
                    pz[:, cg : cg + 1],
                    lhsT=W2t[:, 128 * cg : 128 * cg + 128],
                    rhs=hT[:, b : b + 1],
                    start=True, stop=True,
                )
            nc.scalar.activation(
                out=Zpp[:, 2 * b : 2 * b + 2], in_=pz,
                func=mybir.ActivationFunctionType.Sigmoid,
            )

        def mult_store(j, half, i):
            t = T16[2 * j + half]
            b, cg = j // 2, j % 2
            if i % 2 == 0:
                nc.scalar.mul(out=t, in_=t, mul=Zpp[:, j : j + 1])
            else:
                nc.vector.tensor_tensor(
                    out=t,
                    in0=t,
                    in1=Zpp[:, j : j + 1].broadcast_to([128, 2048]),
                    op=mybir.AluOpType.mult,
                )
            HWE[i % 2].dma_start(
                out=out_v[:, b, cg, 2048 * half : 2048 * (half + 1)],
                in_=t,
            )

        out_v = out_d.rearrange("b (cg cl) h w -> cl b cg (h w)", cg=2)
        stage1(0)
        stage1(1)
        for v in range(8):
            stage2(0, v)
        stage1(2)
        stage1(3)
        phase_b(0)
        for jj in (0, 1):
            for half in range(2):
                mult_store(jj, half, 2 * jj + half)
        for v in range(8):
            stage2(1, v)
        phase_b(1)
        for jj in (2, 3):
            for half in range(2):
                t = T16[2 * jj + half]
                b, cg = jj // 2, jj % 2
                for qq in range(2):
                    i = 4 * jj + 2 * half + qq
                    sl = slice(1024 * qq, 1024 * (qq + 1))
                    if i % 2 == 0:
                        nc.scalar.mul(
                            out=t[:, sl], in_=t[:, sl], mul=Zpp[:, jj : jj + 1]
                        )
                    else:
                        nc.vector.tensor_tensor(
                            out=t[:, sl],
                            in0=t[:, sl],
                            in1=Zpp[:, jj : jj + 1].broadcast_to([128, 1024]),
                            op=mybir.AluOpType.mult,
                        )
                    HWE[i % 2].dma_start(
                        out=out_v[:, b, cg, 2048 * half + 1024 * qq :
                                 2048 * half + 1024 * (qq + 1)],
                        in_=t[:, sl],
                    )

    nc.finalize()
    return nc


def get_nc():
    key = ("nc", CONV_MODE)
    if key not in _NC_CACHE:
        _NC_CACHE[key] = _build_nc(CONV_MODE)
    return _NC_CACHE[key]


def make_in_map(ip_shard, w1, w2):
    BD1, BD2, identf = _constants()
    cp = np.arange(128)
    ch_even = 128 * (cp // 64) + 2 * (cp % 64)
    return {
        "ip": np.ascontiguousarray(ip_shard, dtype=np.float32),
        "w1e": np.ascontiguousarray(w1[ch_even], dtype=np.float32),
        "w1o": np.ascontiguousarray(w1[ch_even + 1], dtype=np.float32),
        "w2": np.ascontiguousarray(w2, dtype=np.float32),
        "bd1": BD1,
        "bd2": BD2,
        "identf": identf,
    }


def kernel(ip, w1, w2):
    assert ip.shape == (B, C, H, W), ip.shape
    nc = get_nc()
    ip = np.ascontiguousarray(ip, dtype=np.float32)
    w1 = np.asarray(w1, dtype=np.float32)
    w2 = np.asarray(w2, dtype=np.float32)
    in_maps = [
        make_in_map(ip[B_LOC * k : B_LOC * (k + 1)], w1, w2)
        for k in range(N_CORES)
    ]
    res = run_bass_kernel_spmd(nc, in_maps, list(range(N_CORES)), **RUN_KWARGS)
    LAST_RESULT.clear()
    LAST_RESULT["exec_time_ns"] = res.exec_time_ns
    LAST_RESULT["profile_json"] = res.profile_json
    return np.concatenate([m["out"] for m in res.results], axis=0)


RUN_KWARGS = {}
LAST_RESULT = {}
